# revision 31
# baseline (speedup 1.0000x reference)
"""Trainium2 Bass kernel for DilateAttention (3x3 kernel, dilation 2).

Computation (see module docstring in the original nn.Module):
  q,k,v: [B=4, d=384, H=64, W=64] f32.  heads=12, head_dim=32.
  For every pixel l and head n: attend over the 9 dilated neighbors
  (offsets {0,2,4} - 2 in each spatial dim, zero-padded).
  out: [B, H, W, d] f32.

Mapping (per NeuronCore; 8 cores; core = (batch b, row-half)):
  - channel-major layout: 128 partitions = 4 heads x 32 channels (3 groups)
  - 9 shifted elementwise q*k products on DVE (bf16)
  - partition reduction (32->1 per head) via block-diagonal ones matmul on
    TensorE, 9 offsets accumulated into one PSUM tile -> logits [36, N]
  - exp on ScalarE (scale folded in), Z via ones matmul, 1/Z on DVE
    (reciprocal_approx_fast), normalize -> attn [36, N] bf16
  - broadcast attn rows back to 128 partitions via selector matmul,
    multiply with shifted V (DVE), accumulate over 9 offsets
  - PE transpose [128,128] blocks -> pixel-major PSUM f32 -> DMA to DRAM
"""

import os
import sys

for _p in ("/opt/trn_rl_repo",):
    if _p not in sys.path and os.path.isdir(_p):
        sys.path.insert(0, _p)

from contextlib import ExitStack

import numpy as np
import ml_dtypes

import concourse.bass as bass
import concourse.bacc as bacc
import concourse.mybir as mybir
import concourse.tile as tile
from concourse import masks
from concourse.bass_utils import run_bass_kernel_spmd

BF16 = ml_dtypes.bfloat16

# problem constants (hardcoded per spec)
B, D, H, W = 4, 384, 64, 64
NH, HD = 12, 32
KK, DIL, PAD = 3, 2, 2
K2 = KK * KK
SCALE = HD ** -0.5

NCORES = 8
ROWS = H // 2              # 32 output rows per core
HROWS = ROWS + 2 * PAD     # 36 halo rows of padded k/v
WP = W + 2 * PAD           # 68 padded width
NGRP = 3                   # channel groups of 128 partitions
HPG = 4                    # heads per group
FD = ROWS * W              # 2048 pixels per core
NCH = 4                    # psum chunks
CHD = FD // NCH            # 512 chunk free dim
LG = HPG * K2              # 36 logit rows per group

_CACHE = {}


def _build_sel_constants():
    """Selector/ones matrices used as TensorE stationary operands."""
    # QK reduce: for offset ko, lhsT[p=hl*32+c, m=hl*9+ko] = 1
    selqk = np.zeros((128, K2, LG), np.float32)
    for hl in range(HPG):
        for c in range(HD):
            for ko in range(K2):
                selqk[hl * HD + c, ko, hl * K2 + ko] = 1.0
    # Z: lhsT[p=hl*9+ko, m=hl] = 1
    selz = np.zeros((LG, HPG), np.float32)
    for hl in range(HPG):
        for ko in range(K2):
            selz[hl * K2 + ko, hl] = 1.0
    # Z broadcast back: lhsT[p=hl, m=hl*9+ko] = 1  (f32; rhs is f32 1/Z)
    selzt = selz.T.copy()
    # attn broadcast: for offset ko, lhsT[p=hl*9+ko, m=hl*32+c] = 1
    selbc = np.zeros((LG, K2, 128), np.float32)
    for hl in range(HPG):
        for ko in range(K2):
            for c in range(HD):
                selbc[hl * K2 + ko, ko, hl * HD + c] = 1.0
    return (
        selqk.reshape(128, K2 * LG).astype(BF16),
        selz.astype(BF16),
        selzt.astype(np.float32),
        selbc.reshape(LG, K2 * 128).astype(BF16),
    )


def _build_nc():
    nc = bacc.Bacc("TRN2", target_bir_lowering=False, debug=False,
                   num_devices=NCORES)
    f32 = mybir.dt.float32
    bf16 = mybir.dt.bfloat16

    q_p = nc.declare_dram_parameter("q", [D, FD], bf16, isOutput=False)
    k_p = nc.declare_dram_parameter("k", [D, HROWS * WP], bf16, isOutput=False)
    v_p = nc.declare_dram_parameter("v", [D, HROWS * WP], bf16, isOutput=False)
    selqk_p = nc.declare_dram_parameter("selqk", [128, K2 * LG], bf16, isOutput=False)
    selz_p = nc.declare_dram_parameter("selz", [LG, HPG], bf16, isOutput=False)
    selzt_p = nc.declare_dram_parameter("selzt", [HPG, LG], f32, isOutput=False)
    selbc_p = nc.declare_dram_parameter("selbc", [LG, K2 * 128], bf16, isOutput=False)
    out_p = nc.declare_dram_parameter("out", [FD, D], f32, isOutput=True)

    with tile.TileContext(nc) as tc, ExitStack() as ctx:
        consts = ctx.enter_context(tc.tile_pool(name="consts", bufs=1))
        inp = ctx.enter_context(tc.tile_pool(name="inp", bufs=2))
        prods = ctx.enter_context(tc.tile_pool(name="prods", bufs=1))
        smax = ctx.enter_context(tc.tile_pool(name="smax", bufs=2))
        accp = ctx.enter_context(tc.tile_pool(name="accp", bufs=2))
        ps_lg = ctx.enter_context(tc.tile_pool(name="ps_lg", bufs=1, space="PSUM"))
        ps_zz = ctx.enter_context(tc.tile_pool(name="ps_zz", bufs=1, space="PSUM"))
        ps_bc = ctx.enter_context(tc.tile_pool(name="ps_bc", bufs=2, space="PSUM"))
        ps_tx = ctx.enter_context(tc.tile_pool(name="ps_tx", bufs=1, space="PSUM"))

        # constants
        selqk_t = consts.tile([128, K2 * LG], bf16)
        nc.sync.dma_start(selqk_t[:], selqk_p[:])
        selz_t = consts.tile([LG, HPG], bf16)
        nc.sync.dma_start(selz_t[:], selz_p[:])
        selzt_t = consts.tile([HPG, LG], f32)
        nc.sync.dma_start(selzt_t[:], selzt_p[:])
        selbc_t = consts.tile([LG, K2 * 128], bf16)
        nc.sync.dma_start(selbc_t[:], selbc_p[:])
        ident = consts.tile([128, 128], bf16)
        masks.make_identity(nc, ident[:])
        identf = consts.tile([4, 4], f32)
        masks.make_identity(nc, identf[:])

        # ~5us of dummy matmuls right at kernel start: drives the PE HAM
        # activity window so the real matmul stream runs at 2.4 GHz.
        warm = ps_lg.tile([LG, CHD], f32, tag="lg", name="warm")
        for _ in range(16):
            nc.tensor.matmul(warm[:, :K2 * LG], selqk_t[:, :LG],
                             selqk_t[:, :K2 * LG], start=True, stop=True)

        def shifted(t3, ko, ch=None):
            """AP into padded [128, HROWS, WP] tile for offset ko, chunk ch."""
            di, dj = divmod(ko, KK)
            rows = slice(ch * (ROWS // NCH), (ch + 1) * (ROWS // NCH)) if ch is not None \
                else slice(0, ROWS)
            r0 = rows.start
            nr = rows.stop - rows.start
            return t3[:, DIL * di + r0: DIL * di + r0 + nr, DIL * dj: DIL * dj + W]

        for g in range(NGRP):
            gp = slice(g * 128, (g + 1) * 128)
            q_t = inp.tile([128, FD], bf16, tag="q")
            nc.sync.dma_start(q_t[:], q_p[gp, :])
            k_t = inp.tile([128, HROWS * WP], bf16, tag="k")
            nc.sync.dma_start(k_t[:], k_p[gp, :])
            v_t = inp.tile([128, HROWS * WP], bf16, tag="v")
            nc.sync.dma_start(v_t[:], v_p[gp, :])
            k3 = k_t[:].rearrange("p (r w) -> p r w", r=HROWS)
            v3 = v_t[:].rearrange("p (r w) -> p r w", r=HROWS)

            # 9 shifted q*k products (bf16; 7 on DVE, 2 on GPSIMD)
            ptiles = []
            for ko in range(K2):
                pt = prods.tile([128, FD], bf16, tag=f"p{ko}")
                eng = nc.gpsimd if ko in (2, 5, 7) else nc.vector
                eng.tensor_mul(
                    pt[:].rearrange("p (r w) -> p r w", r=ROWS),
                    q_t[:].rearrange("p (r w) -> p r w", r=ROWS),
                    shifted(k3, ko),
                )
                ptiles.append(pt)

            exp_t = smax.tile([LG, FD], bf16, tag="exp")
            zr_t = smax.tile([HPG, FD], f32, tag="zr")
            for ch in range(NCH):
                cs = slice(ch * CHD, (ch + 1) * CHD)
                lg = ps_lg.tile([LG, CHD], f32, tag="lg")
                for ko in range(K2):
                    nc.tensor.matmul(
                        lg[:],
                        selqk_t[:, ko * LG:(ko + 1) * LG],
                        ptiles[ko][:, cs],
                        start=(ko == 0),
                        stop=(ko == K2 - 1),
                    )
                # exp(scale * logits) -> bf16
                nc.scalar.activation(
                    exp_t[:, cs], lg[:], mybir.ActivationFunctionType.Exp,
                    scale=float(SCALE),
                )
                zp = ps_zz.tile([LG, CHD], f32, tag="zz", name="zp")
                nc.tensor.matmul(zp[:HPG, :], selz_t[:], exp_t[:, cs],
                                 start=True, stop=True)
                nc.vector.reciprocal_approx_fast(zr_t[:, cs], zp[:HPG, :])
            # transpose 1/Z to pixel-major: zrt[p, t*4+h] = zr[h, t*128+p]
            zrt_ps = ps_zz.tile([128, (FD // 128) * HPG], f32, tag="zz",
                                name="zrt_ps")
            for t in range(FD // 128):
                nc.tensor.matmul(zrt_ps[:, t * HPG:(t + 1) * HPG],
                                 zr_t[:, t * 128:(t + 1) * 128], identf[:],
                                 start=True, stop=True)
            zrt = smax.tile([128, (FD // 128) * HPG], f32, tag="zrt")
            nc.vector.tensor_copy(zrt[:], zrt_ps[:])

            # AV: out[c,l] = sum_k attn_bcast_k * v_shift_k.
            # The sum over k and the channel->pixel transpose both happen on
            # TensorE: each product tile is fed as the stationary operand of
            # an identity matmul accumulating into a pixel-major PSUM tile.
            PCD = 1024           # pixels per AV chunk
            NPC = FD // PCD      # 2 chunks
            RPC = PCD // W       # 16 rows per chunk
            ACT_KO = (0, 1, 3, 4, 6, 8)   # bc copied to SBUF by ScalarE
            for pc in range(NPC):
                pavs = []
                r0 = pc * RPC
                for ko in range(K2):
                    di, dj = divmod(ko, KK)
                    bc = ps_bc.tile([128, PCD], f32, tag="bc")
                    for h in range(PCD // CHD):
                        nc.tensor.matmul(
                            bc[:, h * CHD:(h + 1) * CHD],
                            selbc_t[:, ko * 128:(ko + 1) * 128],
                            exp_t[:, pc * PCD + h * CHD: pc * PCD + (h + 1) * CHD],
                            start=True, stop=True,
                        )
                    pav = accp.tile([128, PCD], bf16, tag=f"pav{ko}",
                                    name=f"pav{ko}")
                    vsh = v3[:, DIL * di + r0: DIL * di + r0 + RPC,
                             DIL * dj: DIL * dj + W]
                    if ko in ACT_KO:
                        bcs = smax.tile([128, PCD], bf16, tag="bcs")
                        nc.scalar.copy(bcs[:], bc[:])
                        nc.vector.tensor_mul(
                            pav[:].rearrange("p (r w) -> p r w", r=RPC),
                            bcs[:].rearrange("p (r w) -> p r w", r=RPC), vsh)
                    else:
                        nc.vector.tensor_mul(
                            pav[:].rearrange("p (r w) -> p r w", r=RPC),
                            bc[:].rearrange("p (r w) -> p r w", r=RPC), vsh)
                    pavs.append(pav)
                # sum over k: pavs 5..8 pair-added on DVE (bf16 2x), the rest
                # accumulated on TensorE (identity stationary, PSUM)
                s56 = accp.tile([128, PCD], bf16, tag="s56")
                nc.vector.tensor_add(s56[:], pavs[5][:], pavs[6][:])
                s78 = accp.tile([128, PCD], bf16, tag="s78")
                nc.vector.tensor_add(s78[:], pavs[7][:], pavs[8][:])
                s58 = accp.tile([128, PCD], bf16, tag="s58")
                nc.vector.tensor_add(s58[:], s56[:], s78[:])
                terms = [pavs[0], pavs[1], pavs[2], pavs[3], pavs[4], s58]
                acc_ps = ps_tx.tile([128, PCD], f32, tag="tx", name="acc_ps")
                for h in range(PCD // CHD):
                    hs = slice(h * CHD, (h + 1) * CHD)
                    for i, tm in enumerate(terms):
                        nc.tensor.matmul(acc_ps[:, hs], ident[:], tm[:, hs],
                                         start=(i == 0),
                                         stop=(i == len(terms) - 1))
                accs = smax.tile([128, PCD], bf16, tag="accs")
                nc.vector.tensor_copy(accs[:], acc_ps[:])
                # transpose to pixel-major
                tp = ps_tx.tile([128, PCD], f32, tag="tx", name="tp")
                for j in range(PCD // 128):
                    js = slice(j * 128, (j + 1) * 128)
                    nc.tensor.matmul(tp[:, js], accs[:, js], ident[:],
                                     start=True, stop=True)
                # fused normalization: st[p, j, h, c] = tp * zr_t(pixel-major)
                st = smax.tile([128, PCD], f32, tag="st")
                nj = PCD // 128
                zfac = zrt[:, pc * nj * HPG:(pc + 1) * nj * HPG].rearrange(
                    "p (j h) -> p j h", j=nj).broadcast_to((128, nj, HPG, HD))
                nc.vector.tensor_mul(
                    st[:].rearrange("p (j h c) -> p j h c", j=nj, h=HPG),
                    tp[:].rearrange("p (j h c) -> p j h c", j=nj, h=HPG),
                    zfac,
                )
                dst = out_p[pc * PCD:(pc + 1) * PCD, gp].rearrange(
                    "(j p) d -> p j d", p=128)
                nc.sync.dma_start(dst, st[:].rearrange("p (j d) -> p j d",
                                                       j=PCD // 128))

    nc.compile()
    return nc


def _get_nc():
    if "nc" not in _CACHE:
        _CACHE["nc"] = _build_nc()
    return _CACHE["nc"]


def kernel(q, k, v):
    q = np.asarray(q, dtype=np.float32)
    k = np.asarray(k, dtype=np.float32)
    v = np.asarray(v, dtype=np.float32)

    qb = q.astype(BF16)
    kp = np.pad(k, ((0, 0), (0, 0), (PAD, PAD), (PAD, PAD))).astype(BF16)
    vp = np.pad(v, ((0, 0), (0, 0), (PAD, PAD), (PAD, PAD))).astype(BF16)

    selqk, selz, selzt, selbc = _CACHE.setdefault("sel", _build_sel_constants())

    in_maps = []
    for c in range(NCORES):
        b, half = divmod(c, 2)
        r0 = half * ROWS
        in_maps.append({
            "q": np.ascontiguousarray(qb[b, :, r0:r0 + ROWS, :]).reshape(D, FD),
            "k": np.ascontiguousarray(kp[b, :, r0:r0 + HROWS, :]).reshape(D, HROWS * WP),
            "v": np.ascontiguousarray(vp[b, :, r0:r0 + HROWS, :]).reshape(D, HROWS * WP),
            "selqk": selqk, "selz": selz, "selzt": selzt, "selbc": selbc,
        })

    nc = _get_nc()
    res = run_bass_kernel_spmd(nc, in_maps, core_ids=list(range(NCORES)))

    out = np.empty((B, H, W, D), np.float32)
    for c in range(NCORES):
        b, half = divmod(c, 2)
        r0 = half * ROWS
        out[b, r0:r0 + ROWS] = res.results[c]["out"].reshape(ROWS, W, D)
    return out


# revision 32
# speedup vs baseline: 1.0425x; 1.0425x over previous
"""Trainium2 Bass kernel for DilateAttention (3x3 kernel, dilation 2).

Computation (see module docstring in the original nn.Module):
  q,k,v: [B=4, d=384, H=64, W=64] f32.  heads=12, head_dim=32.
  For every pixel l and head n: attend over the 9 dilated neighbors
  (offsets {0,2,4} - 2 in each spatial dim, zero-padded).
  out: [B, H, W, d] f32.

Mapping (per NeuronCore; 8 cores; core = (batch b, row-half)):
  - channel-major layout: 128 partitions = 4 heads x 32 channels (3 groups)
  - 9 shifted elementwise q*k products on DVE (bf16)
  - partition reduction (32->1 per head) via block-diagonal ones matmul on
    TensorE, 9 offsets accumulated into one PSUM tile -> logits [36, N]
  - exp on ScalarE (scale folded in), Z via ones matmul, 1/Z on DVE
    (reciprocal_approx_fast), normalize -> attn [36, N] bf16
  - broadcast attn rows back to 128 partitions via selector matmul,
    multiply with shifted V (DVE), accumulate over 9 offsets
  - PE transpose [128,128] blocks -> pixel-major PSUM f32 -> DMA to DRAM
"""

import os
import sys

for _p in ("/opt/trn_rl_repo",):
    if _p not in sys.path and os.path.isdir(_p):
        sys.path.insert(0, _p)

from contextlib import ExitStack

import numpy as np
import ml_dtypes

import concourse.bass as bass
import concourse.bacc as bacc
import concourse.mybir as mybir
import concourse.tile as tile
from concourse import masks
from concourse.bass_utils import run_bass_kernel_spmd

BF16 = ml_dtypes.bfloat16

# problem constants (hardcoded per spec)
B, D, H, W = 4, 384, 64, 64
NH, HD = 12, 32
KK, DIL, PAD = 3, 2, 2
K2 = KK * KK
SCALE = HD ** -0.5

NCORES = 8
ROWS = H // 2              # 32 output rows per core
HROWS = ROWS + 2 * PAD     # 36 halo rows of padded k/v
WP = W + 2 * PAD           # 68 padded width
NGRP = 3                   # channel groups of 128 partitions
HPG = 4                    # heads per group
FD = ROWS * W              # 2048 pixels per core
NCH = 4                    # psum chunks
CHD = FD // NCH            # 512 chunk free dim
LG = HPG * K2              # 36 logit rows per group

_CACHE = {}


def _build_sel_constants():
    """Selector/ones matrices used as TensorE stationary operands."""
    # QK reduce: for offset ko, lhsT[p=hl*32+c, m=hl*9+ko] = 1
    selqk = np.zeros((128, K2, LG), np.float32)
    for hl in range(HPG):
        for c in range(HD):
            for ko in range(K2):
                selqk[hl * HD + c, ko, hl * K2 + ko] = 1.0
    # Z: lhsT[p=hl*9+ko, m=hl] = 1
    selz = np.zeros((LG, HPG), np.float32)
    for hl in range(HPG):
        for ko in range(K2):
            selz[hl * K2 + ko, hl] = 1.0
    # Z broadcast back: lhsT[p=hl, m=hl*9+ko] = 1  (f32; rhs is f32 1/Z)
    selzt = selz.T.copy()
    # attn broadcast: for offset ko, lhsT[p=hl*9+ko, m=hl*32+c] = 1
    selbc = np.zeros((LG, K2, 128), np.float32)
    for hl in range(HPG):
        for ko in range(K2):
            for c in range(HD):
                selbc[hl * K2 + ko, ko, hl * HD + c] = 1.0
    return (
        selqk.reshape(128, K2 * LG).astype(BF16),
        selz.astype(BF16),
        selzt.astype(np.float32),
        selbc.reshape(LG, K2 * 128).astype(BF16),
    )


def _build_nc():
    nc = bacc.Bacc("TRN2", target_bir_lowering=False, debug=False,
                   num_devices=NCORES)
    f32 = mybir.dt.float32
    bf16 = mybir.dt.bfloat16

    q_p = nc.declare_dram_parameter("q", [D, FD], bf16, isOutput=False)
    k_p = nc.declare_dram_parameter("k", [D, HROWS * WP], bf16, isOutput=False)
    v_p = nc.declare_dram_parameter("v", [D, HROWS * WP], bf16, isOutput=False)
    selqk_p = nc.declare_dram_parameter("selqk", [128, K2 * LG], bf16, isOutput=False)
    selz_p = nc.declare_dram_parameter("selz", [LG, HPG], bf16, isOutput=False)
    selzt_p = nc.declare_dram_parameter("selzt", [HPG, LG], f32, isOutput=False)
    selbc_p = nc.declare_dram_parameter("selbc", [LG, K2 * 128], bf16, isOutput=False)
    out_p = nc.declare_dram_parameter("out", [FD, D], f32, isOutput=True)

    with tile.TileContext(nc) as tc, ExitStack() as ctx:
        consts = ctx.enter_context(tc.tile_pool(name="consts", bufs=1))
        inp = ctx.enter_context(tc.tile_pool(name="inp", bufs=2))
        prods = ctx.enter_context(tc.tile_pool(name="prods", bufs=1))
        smax = ctx.enter_context(tc.tile_pool(name="smax", bufs=2))
        accp = ctx.enter_context(tc.tile_pool(name="accp", bufs=2))
        ps_lg = ctx.enter_context(tc.tile_pool(name="ps_lg", bufs=1, space="PSUM"))
        ps_zz = ctx.enter_context(tc.tile_pool(name="ps_zz", bufs=1, space="PSUM"))
        ps_bc = ctx.enter_context(tc.tile_pool(name="ps_bc", bufs=2, space="PSUM"))
        ps_tx = ctx.enter_context(tc.tile_pool(name="ps_tx", bufs=1, space="PSUM"))

        # constants
        selqk_t = consts.tile([128, K2 * LG], bf16)
        nc.sync.dma_start(selqk_t[:], selqk_p[:])
        selz_t = consts.tile([LG, HPG], bf16)
        nc.sync.dma_start(selz_t[:], selz_p[:])
        selzt_t = consts.tile([HPG, LG], f32)
        nc.sync.dma_start(selzt_t[:], selzt_p[:])
        selbc_t = consts.tile([LG, K2 * 128], bf16)
        nc.sync.dma_start(selbc_t[:], selbc_p[:])
        ident = consts.tile([128, 128], bf16)
        masks.make_identity(nc, ident[:])
        identf = consts.tile([4, 4], f32)
        masks.make_identity(nc, identf[:])

        # ~5us of dummy matmuls right at kernel start: drives the PE HAM
        # activity window so the real matmul stream runs at 2.4 GHz.
        warm = ps_lg.tile([LG, CHD], f32, tag="lg", name="warm")
        for _ in range(16):
            nc.tensor.matmul(warm[:, :K2 * LG], selqk_t[:, :LG],
                             selqk_t[:, :K2 * LG], start=True, stop=True)

        def shifted(t3, ko, ch=None):
            """AP into padded [128, HROWS, WP] tile for offset ko, chunk ch."""
            di, dj = divmod(ko, KK)
            rows = slice(ch * (ROWS // NCH), (ch + 1) * (ROWS // NCH)) if ch is not None \
                else slice(0, ROWS)
            r0 = rows.start
            nr = rows.stop - rows.start
            return t3[:, DIL * di + r0: DIL * di + r0 + nr, DIL * dj: DIL * dj + W]

        for g in range(NGRP):
            gp = slice(g * 128, (g + 1) * 128)
            q_t = inp.tile([128, FD], bf16, tag="q")
            nc.sync.dma_start(q_t[:], q_p[gp, :])
            k_t = inp.tile([128, HROWS * WP], bf16, tag="k")
            nc.sync.dma_start(k_t[:], k_p[gp, :])
            v_t = inp.tile([128, HROWS * WP], bf16, tag="v")
            nc.sync.dma_start(v_t[:], v_p[gp, :])
            k3 = k_t[:].rearrange("p (r w) -> p r w", r=HROWS)
            v3 = v_t[:].rearrange("p (r w) -> p r w", r=HROWS)

            # 9 shifted q*k products (bf16; 7 on DVE, 2 on GPSIMD)
            ptiles = []
            for ko in range(K2):
                pt = prods.tile([128, FD], bf16, tag=f"p{ko}")
                eng = nc.gpsimd if ko in (2, 5, 7) else nc.vector
                eng.tensor_mul(
                    pt[:].rearrange("p (r w) -> p r w", r=ROWS),
                    q_t[:].rearrange("p (r w) -> p r w", r=ROWS),
                    shifted(k3, ko),
                )
                ptiles.append(pt)

            exp_t = smax.tile([LG, FD], bf16, tag="exp")
            zr_t = smax.tile([HPG, FD], f32, tag="zr")
            for ch in range(NCH):
                cs = slice(ch * CHD, (ch + 1) * CHD)
                lg = ps_lg.tile([LG, CHD], f32, tag="lg")
                for ko in range(K2):
                    nc.tensor.matmul(
                        lg[:],
                        selqk_t[:, ko * LG:(ko + 1) * LG],
                        ptiles[ko][:, cs],
                        start=(ko == 0),
                        stop=(ko == K2 - 1),
                    )
                # exp(scale * logits) -> bf16
                nc.scalar.activation(
                    exp_t[:, cs], lg[:], mybir.ActivationFunctionType.Exp,
                    scale=float(SCALE),
                )
                zp = ps_zz.tile([LG, CHD], f32, tag="zz", name="zp")
                nc.tensor.matmul(zp[:HPG, :], selz_t[:], exp_t[:, cs],
                                 start=True, stop=True)
                nc.vector.reciprocal_approx_fast(zr_t[:, cs], zp[:HPG, :])
            # transpose 1/Z to pixel-major: zrt[p, t*4+h] = zr[h, t*128+p]
            zrt_ps = ps_zz.tile([128, (FD // 128) * HPG], f32, tag="zz",
                                name="zrt_ps")
            for t in range(FD // 128):
                nc.tensor.matmul(zrt_ps[:, t * HPG:(t + 1) * HPG],
                                 zr_t[:, t * 128:(t + 1) * 128], identf[:],
                                 start=True, stop=True)
            zrt = smax.tile([128, (FD // 128) * HPG], f32, tag="zrt")
            nc.vector.tensor_copy(zrt[:], zrt_ps[:])

            # AV: out[c,l] = sum_k attn_bcast_k * v_shift_k.
            # The sum over k and the channel->pixel transpose both happen on
            # TensorE: each product tile is fed as the stationary operand of
            # an identity matmul accumulating into a pixel-major PSUM tile.
            PCD = 1024           # pixels per AV chunk
            NPC = FD // PCD      # 2 chunks
            RPC = PCD // W       # 16 rows per chunk
            ACT_KO = (0, 1, 3, 4, 6, 8)   # bc copied to SBUF by ScalarE
            for pc in range(NPC):
                pavs = []
                r0 = pc * RPC
                for ko in range(K2):
                    di, dj = divmod(ko, KK)
                    bc = ps_bc.tile([128, PCD], f32, tag="bc")
                    for h in range(PCD // CHD):
                        nc.tensor.matmul(
                            bc[:, h * CHD:(h + 1) * CHD],
                            selbc_t[:, ko * 128:(ko + 1) * 128],
                            exp_t[:, pc * PCD + h * CHD: pc * PCD + (h + 1) * CHD],
                            start=True, stop=True,
                        )
                    pav = accp.tile([128, PCD], bf16, tag=f"pav{ko}",
                                    name=f"pav{ko}")
                    vsh = v3[:, DIL * di + r0: DIL * di + r0 + RPC,
                             DIL * dj: DIL * dj + W]
                    if ko in ACT_KO:
                        bcs = smax.tile([128, PCD], bf16, tag="bcs")
                        nc.scalar.copy(bcs[:], bc[:])
                        nc.vector.tensor_mul(
                            pav[:].rearrange("p (r w) -> p r w", r=RPC),
                            bcs[:].rearrange("p (r w) -> p r w", r=RPC), vsh)
                    else:
                        nc.vector.tensor_mul(
                            pav[:].rearrange("p (r w) -> p r w", r=RPC),
                            bc[:].rearrange("p (r w) -> p r w", r=RPC), vsh)
                    pavs.append(pav)
                # sum over k on TensorE (identity stationary, PSUM accumulate)
                acc_ps = ps_tx.tile([128, PCD], f32, tag="tx", name="acc_ps")
                for h in range(PCD // CHD):
                    hs = slice(h * CHD, (h + 1) * CHD)
                    for ko in range(K2):
                        nc.tensor.matmul(acc_ps[:, hs], ident[:],
                                         pavs[ko][:, hs],
                                         start=(ko == 0), stop=(ko == K2 - 1))
                accs = smax.tile([128, PCD], bf16, tag="accs")
                nc.vector.tensor_copy(accs[:], acc_ps[:])
                # transpose to pixel-major
                tp = ps_tx.tile([128, PCD], f32, tag="tx", name="tp")
                for j in range(PCD // 128):
                    js = slice(j * 128, (j + 1) * 128)
                    nc.tensor.matmul(tp[:, js], accs[:, js], ident[:],
                                     start=True, stop=True)
                # fused normalization: st[p, j, h, c] = tp * zr_t(pixel-major)
                st = smax.tile([128, PCD], f32, tag="st")
                nj = PCD // 128
                zfac = zrt[:, pc * nj * HPG:(pc + 1) * nj * HPG].rearrange(
                    "p (j h) -> p j h", j=nj).broadcast_to((128, nj, HPG, HD))
                nc.vector.tensor_mul(
                    st[:].rearrange("p (j h c) -> p j h c", j=nj, h=HPG),
                    tp[:].rearrange("p (j h c) -> p j h c", j=nj, h=HPG),
                    zfac,
                )
                dst = out_p[pc * PCD:(pc + 1) * PCD, gp].rearrange(
                    "(j p) d -> p j d", p=128)
                nc.sync.dma_start(dst, st[:].rearrange("p (j d) -> p j d",
                                                       j=PCD // 128))

    nc.compile()
    return nc


def _get_nc():
    if "nc" not in _CACHE:
        _CACHE["nc"] = _build_nc()
    return _CACHE["nc"]


def kernel(q, k, v):
    q = np.asarray(q, dtype=np.float32)
    k = np.asarray(k, dtype=np.float32)
    v = np.asarray(v, dtype=np.float32)

    qb = q.astype(BF16)
    kp = np.pad(k, ((0, 0), (0, 0), (PAD, PAD), (PAD, PAD))).astype(BF16)
    vp = np.pad(v, ((0, 0), (0, 0), (PAD, PAD), (PAD, PAD))).astype(BF16)

    selqk, selz, selzt, selbc = _CACHE.setdefault("sel", _build_sel_constants())

    in_maps = []
    for c in range(NCORES):
        b, half = divmod(c, 2)
        r0 = half * ROWS
        in_maps.append({
            "q": np.ascontiguousarray(qb[b, :, r0:r0 + ROWS, :]).reshape(D, FD),
            "k": np.ascontiguousarray(kp[b, :, r0:r0 + HROWS, :]).reshape(D, HROWS * WP),
            "v": np.ascontiguousarray(vp[b, :, r0:r0 + HROWS, :]).reshape(D, HROWS * WP),
            "selqk": selqk, "selz": selz, "selzt": selzt, "selbc": selbc,
        })

    nc = _get_nc()
    res = run_bass_kernel_spmd(nc, in_maps, core_ids=list(range(NCORES)))

    out = np.empty((B, H, W, D), np.float32)
    for c in range(NCORES):
        b, half = divmod(c, 2)
        r0 = half * ROWS
        out[b, r0:r0 + ROWS] = res.results[c]["out"].reshape(ROWS, W, D)
    return out


# revision 34
# speedup vs baseline: 1.0599x; 1.0166x over previous
"""Trainium2 Bass kernel for DilateAttention (3x3 kernel, dilation 2).

Computation (see module docstring in the original nn.Module):
  q,k,v: [B=4, d=384, H=64, W=64] f32.  heads=12, head_dim=32.
  For every pixel l and head n: attend over the 9 dilated neighbors
  (offsets {0,2,4} - 2 in each spatial dim, zero-padded).
  out: [B, H, W, d] f32.

Mapping (per NeuronCore; 8 cores; core = (batch b, row-half)):
  - channel-major layout: 128 partitions = 4 heads x 32 channels (3 groups)
  - 9 shifted elementwise q*k products on DVE (bf16)
  - partition reduction (32->1 per head) via block-diagonal ones matmul on
    TensorE, 9 offsets accumulated into one PSUM tile -> logits [36, N]
  - exp on ScalarE (scale folded in), Z via ones matmul, 1/Z on DVE
    (reciprocal_approx_fast), normalize -> attn [36, N] bf16
  - broadcast attn rows back to 128 partitions via selector matmul,
    multiply with shifted V (DVE), accumulate over 9 offsets
  - PE transpose [128,128] blocks -> pixel-major PSUM f32 -> DMA to DRAM
"""

import os
import sys

for _p in ("/opt/trn_rl_repo",):
    if _p not in sys.path and os.path.isdir(_p):
        sys.path.insert(0, _p)

from contextlib import ExitStack

import numpy as np
import ml_dtypes

import concourse.bass as bass
import concourse.bacc as bacc
import concourse.mybir as mybir
import concourse.tile as tile
from concourse import masks
from concourse.bass_utils import run_bass_kernel_spmd

BF16 = ml_dtypes.bfloat16

# problem constants (hardcoded per spec)
B, D, H, W = 4, 384, 64, 64
NH, HD = 12, 32
KK, DIL, PAD = 3, 2, 2
K2 = KK * KK
SCALE = HD ** -0.5

NCORES = 8
ROWS = H // 2              # 32 output rows per core
HROWS = ROWS + 2 * PAD     # 36 halo rows of padded k/v
WP = W + 2 * PAD           # 68 padded width
NGRP = 3                   # channel groups of 128 partitions
HPG = 4                    # heads per group
FD = ROWS * W              # 2048 pixels per core
NCH = 4                    # psum chunks
CHD = FD // NCH            # 512 chunk free dim
LG = HPG * K2              # 36 logit rows per group

_CACHE = {}


def _build_sel_constants():
    """Selector/ones matrices used as TensorE stationary operands."""
    # QK reduce: for offset ko, lhsT[p=hl*32+c, m=hl*9+ko] = 1
    selqk = np.zeros((128, K2, LG), np.float32)
    for hl in range(HPG):
        for c in range(HD):
            for ko in range(K2):
                selqk[hl * HD + c, ko, hl * K2 + ko] = 1.0
    # Z: lhsT[p=hl*9+ko, m=hl] = 1
    selz = np.zeros((LG, HPG), np.float32)
    for hl in range(HPG):
        for ko in range(K2):
            selz[hl * K2 + ko, hl] = 1.0
    # Z broadcast back: lhsT[p=hl, m=hl*9+ko] = 1  (f32; rhs is f32 1/Z)
    selzt = selz.T.copy()
    # attn broadcast: for offset ko, lhsT[p=hl*9+ko, m=hl*32+c] = 1
    selbc = np.zeros((LG, K2, 128), np.float32)
    for hl in range(HPG):
        for ko in range(K2):
            for c in range(HD):
                selbc[hl * K2 + ko, ko, hl * HD + c] = 1.0
    return (
        selqk.reshape(128, K2 * LG).astype(BF16),
        selz.astype(BF16),
        selzt.astype(np.float32),
        selbc.reshape(LG, K2 * 128).astype(BF16),
    )


def _build_nc():
    nc = bacc.Bacc("TRN2", target_bir_lowering=False, debug=False,
                   num_devices=NCORES)
    f32 = mybir.dt.float32
    bf16 = mybir.dt.bfloat16

    q_p = nc.declare_dram_parameter("q", [D, FD], bf16, isOutput=False)
    k_p = nc.declare_dram_parameter("k", [D, HROWS * WP], bf16, isOutput=False)
    v_p = nc.declare_dram_parameter("v", [D, HROWS * WP], bf16, isOutput=False)
    selqk_p = nc.declare_dram_parameter("selqk", [128, K2 * LG], bf16, isOutput=False)
    selz_p = nc.declare_dram_parameter("selz", [LG, HPG], bf16, isOutput=False)
    selzt_p = nc.declare_dram_parameter("selzt", [HPG, LG], f32, isOutput=False)
    selbc_p = nc.declare_dram_parameter("selbc", [LG, K2 * 128], bf16, isOutput=False)
    out_p = nc.declare_dram_parameter("out", [FD, D], f32, isOutput=True)

    with tile.TileContext(nc) as tc, ExitStack() as ctx:
        consts = ctx.enter_context(tc.tile_pool(name="consts", bufs=1))
        inp = ctx.enter_context(tc.tile_pool(name="inp", bufs=2))
        prods = ctx.enter_context(tc.tile_pool(name="prods", bufs=1))
        smax = ctx.enter_context(tc.tile_pool(name="smax", bufs=2))
        accp = ctx.enter_context(tc.tile_pool(name="accp", bufs=2))
        ps_lg = ctx.enter_context(tc.tile_pool(name="ps_lg", bufs=1, space="PSUM"))
        ps_zz = ctx.enter_context(tc.tile_pool(name="ps_zz", bufs=1, space="PSUM"))
        ps_bc = ctx.enter_context(tc.tile_pool(name="ps_bc", bufs=2, space="PSUM"))
        ps_tx = ctx.enter_context(tc.tile_pool(name="ps_tx", bufs=1, space="PSUM"))

        # constants
        selqk_t = consts.tile([128, K2 * LG], bf16)
        nc.sync.dma_start(selqk_t[:], selqk_p[:])
        selz_t = consts.tile([LG, HPG], bf16)
        nc.sync.dma_start(selz_t[:], selz_p[:])
        selzt_t = consts.tile([HPG, LG], f32)
        nc.sync.dma_start(selzt_t[:], selzt_p[:])
        selbc_t = consts.tile([LG, K2 * 128], bf16)
        nc.sync.dma_start(selbc_t[:], selbc_p[:])
        ident = consts.tile([128, 128], bf16)
        masks.make_identity(nc, ident[:])
        identf = consts.tile([4, 4], bf16)
        masks.make_identity(nc, identf[:])

        def shifted(t3, ko, ch=None):
            """AP into padded [128, HROWS, WP] tile for offset ko, chunk ch."""
            di, dj = divmod(ko, KK)
            rows = slice(ch * (ROWS // NCH), (ch + 1) * (ROWS // NCH)) if ch is not None \
                else slice(0, ROWS)
            r0 = rows.start
            nr = rows.stop - rows.start
            return t3[:, DIL * di + r0: DIL * di + r0 + nr, DIL * dj: DIL * dj + W]

        for g in range(NGRP):
            gp = slice(g * 128, (g + 1) * 128)
            q_t = inp.tile([128, FD], bf16, tag="q")
            nc.sync.dma_start(q_t[:], q_p[gp, :])
            k_t = inp.tile([128, HROWS * WP], bf16, tag="k")
            nc.sync.dma_start(k_t[:], k_p[gp, :])
            v_t = inp.tile([128, HROWS * WP], bf16, tag="v")
            nc.sync.dma_start(v_t[:], v_p[gp, :])
            k3 = k_t[:].rearrange("p (r w) -> p r w", r=HROWS)
            v3 = v_t[:].rearrange("p (r w) -> p r w", r=HROWS)

            # 9 shifted q*k products (bf16; 7 on DVE, 2 on GPSIMD)
            ptiles = []
            for ko in range(K2):
                pt = prods.tile([128, FD], bf16, tag=f"p{ko}")
                eng = nc.gpsimd if ko in (2, 5, 7) else nc.vector
                eng.tensor_mul(
                    pt[:].rearrange("p (r w) -> p r w", r=ROWS),
                    q_t[:].rearrange("p (r w) -> p r w", r=ROWS),
                    shifted(k3, ko),
                )
                ptiles.append(pt)

            exp_t = smax.tile([LG, FD], bf16, tag="exp")
            zr_t = smax.tile([HPG, FD], f32, tag="zr")
            for ch in range(NCH):
                cs = slice(ch * CHD, (ch + 1) * CHD)
                lg = ps_lg.tile([LG, CHD], f32, tag="lg")
                for ko in range(K2):
                    nc.tensor.matmul(
                        lg[:],
                        selqk_t[:, ko * LG:(ko + 1) * LG],
                        ptiles[ko][:, cs],
                        start=(ko == 0),
                        stop=(ko == K2 - 1),
                    )
                # exp(scale * logits) -> bf16
                nc.scalar.activation(
                    exp_t[:, cs], lg[:], mybir.ActivationFunctionType.Exp,
                    scale=float(SCALE),
                )
                zp = ps_zz.tile([LG, CHD], f32, tag="zz", name="zp")
                nc.tensor.matmul(zp[:HPG, :], selz_t[:], exp_t[:, cs],
                                 start=True, stop=True)
                nc.vector.reciprocal_approx_fast(zr_t[:, cs], zp[:HPG, :])
            # transpose 1/Z to pixel-major: zrt[p, t*4+h] = zr[h, t*128+p]
            zrb = smax.tile([HPG, FD], bf16, tag="zrb")
            nc.vector.tensor_copy(zrb[:], zr_t[:])
            zrt_ps = ps_zz.tile([128, (FD // 128) * HPG], f32, tag="zz",
                                name="zrt_ps")
            for t in range(FD // 128):
                nc.tensor.matmul(zrt_ps[:, t * HPG:(t + 1) * HPG],
                                 zrb[:, t * 128:(t + 1) * 128], identf[:],
                                 start=True, stop=True)
            zrt = smax.tile([128, (FD // 128) * HPG], f32, tag="zrt")
            nc.vector.tensor_copy(zrt[:], zrt_ps[:])

            # AV: out[c,l] = sum_k attn_bcast_k * v_shift_k.
            # The sum over k and the channel->pixel transpose both happen on
            # TensorE: each product tile is fed as the stationary operand of
            # an identity matmul accumulating into a pixel-major PSUM tile.
            PCD = 1024           # pixels per AV chunk
            NPC = FD // PCD      # 2 chunks
            RPC = PCD // W       # 16 rows per chunk
            ACT_KO = (0, 1, 3, 4, 6, 8)   # bc copied to SBUF by ScalarE
            for pc in range(NPC):
                pavs = []
                r0 = pc * RPC
                for ko in range(K2):
                    di, dj = divmod(ko, KK)
                    bc = ps_bc.tile([128, PCD], f32, tag="bc")
                    for h in range(PCD // CHD):
                        nc.tensor.matmul(
                            bc[:, h * CHD:(h + 1) * CHD],
                            selbc_t[:, ko * 128:(ko + 1) * 128],
                            exp_t[:, pc * PCD + h * CHD: pc * PCD + (h + 1) * CHD],
                            start=True, stop=True,
                        )
                    pav = accp.tile([128, PCD], bf16, tag=f"pav{ko}",
                                    name=f"pav{ko}")
                    vsh = v3[:, DIL * di + r0: DIL * di + r0 + RPC,
                             DIL * dj: DIL * dj + W]
                    if ko in ACT_KO:
                        bcs = smax.tile([128, PCD], bf16, tag="bcs")
                        nc.scalar.copy(bcs[:], bc[:])
                        nc.vector.tensor_mul(
                            pav[:].rearrange("p (r w) -> p r w", r=RPC),
                            bcs[:].rearrange("p (r w) -> p r w", r=RPC), vsh)
                    else:
                        nc.vector.tensor_mul(
                            pav[:].rearrange("p (r w) -> p r w", r=RPC),
                            bc[:].rearrange("p (r w) -> p r w", r=RPC), vsh)
                    pavs.append(pav)
                # sum over k on TensorE (identity stationary, PSUM accumulate)
                acc_ps = ps_tx.tile([128, PCD], f32, tag="tx", name="acc_ps")
                for h in range(PCD // CHD):
                    hs = slice(h * CHD, (h + 1) * CHD)
                    for ko in range(K2):
                        nc.tensor.matmul(acc_ps[:, hs], ident[:],
                                         pavs[ko][:, hs],
                                         start=(ko == 0), stop=(ko == K2 - 1))
                accs = smax.tile([128, PCD], bf16, tag="accs")
                nc.vector.tensor_copy(accs[:], acc_ps[:])
                # transpose to pixel-major
                tp = ps_tx.tile([128, PCD], f32, tag="tx", name="tp")
                for j in range(PCD // 128):
                    js = slice(j * 128, (j + 1) * 128)
                    nc.tensor.matmul(tp[:, js], accs[:, js], ident[:],
                                     start=True, stop=True)
                # fused normalization: st[p, j, h, c] = tp * zr_t(pixel-major)
                st = smax.tile([128, PCD], f32, tag="st")
                nj = PCD // 128
                zfac = zrt[:, pc * nj * HPG:(pc + 1) * nj * HPG].rearrange(
                    "p (j h) -> p j h", j=nj).broadcast_to((128, nj, HPG, HD))
                nc.vector.tensor_mul(
                    st[:].rearrange("p (j h c) -> p j h c", j=nj, h=HPG),
                    tp[:].rearrange("p (j h c) -> p j h c", j=nj, h=HPG),
                    zfac,
                )
                dst = out_p[pc * PCD:(pc + 1) * PCD, gp].rearrange(
                    "(j p) d -> p j d", p=128)
                nc.sync.dma_start(dst, st[:].rearrange("p (j d) -> p j d",
                                                       j=PCD // 128))

    nc.compile()
    return nc


def _get_nc():
    if "nc" not in _CACHE:
        _CACHE["nc"] = _build_nc()
    return _CACHE["nc"]


def kernel(q, k, v):
    q = np.asarray(q, dtype=np.float32)
    k = np.asarray(k, dtype=np.float32)
    v = np.asarray(v, dtype=np.float32)

    qb = q.astype(BF16)
    kp = np.pad(k, ((0, 0), (0, 0), (PAD, PAD), (PAD, PAD))).astype(BF16)
    vp = np.pad(v, ((0, 0), (0, 0), (PAD, PAD), (PAD, PAD))).astype(BF16)

    selqk, selz, selzt, selbc = _CACHE.setdefault("sel", _build_sel_constants())

    in_maps = []
    for c in range(NCORES):
        b, half = divmod(c, 2)
        r0 = half * ROWS
        in_maps.append({
            "q": np.ascontiguousarray(qb[b, :, r0:r0 + ROWS, :]).reshape(D, FD),
            "k": np.ascontiguousarray(kp[b, :, r0:r0 + HROWS, :]).reshape(D, HROWS * WP),
            "v": np.ascontiguousarray(vp[b, :, r0:r0 + HROWS, :]).reshape(D, HROWS * WP),
            "selqk": selqk, "selz": selz, "selzt": selzt, "selbc": selbc,
        })

    nc = _get_nc()
    res = run_bass_kernel_spmd(nc, in_maps, core_ids=list(range(NCORES)))

    out = np.empty((B, H, W, D), np.float32)
    for c in range(NCORES):
        b, half = divmod(c, 2)
        r0 = half * ROWS
        out[b, r0:r0 + ROWS] = res.results[c]["out"].reshape(ROWS, W, D)
    return out
